# revision 25
# baseline (speedup 1.0000x reference)
import numpy as np

import concourse.bass as bass
import concourse.mybir as mybir
import concourse.tile as tile
from concourse import bacc
from concourse.bass_utils import run_bass_kernel_spmd

D = 1024
B = 16384
NCORES = 8
BS = B // NCORES
PT = 128
MT = BS // PT
KT = D // PT
NOUT = 3 * D
NFREE = 512
NT = NOUT // NFREE

MM_DT = mybir.dt.float16
NP_DT = np.float16

_COMPILED = None
LAST_RESULTS = None

P0 = ((0, 2, "pool"), (2, 4, "act"), (4, 5, "pool"), (5, 6, "sp"))
W_WHOLE_Q = ((1, "act"), (2, "pool"), (3, "sp"), (4, "act"),
             (5, "pool"), (6, "act"), (7, "sp"))


def _build(repeat=1, timing=False, chunked_w=True, split_tail=True):
    nc = bacc.Bacc("TRN2", target_bir_lowering=False, debug=False,
                   num_devices=NCORES)
    if timing:
        st = nc.dram_tensor("st", [PT, KT * PT], MM_DT, kind="ExternalInput")
        w = nc.dram_tensor("w", [KT, PT, NOUT], MM_DT)
        bi = nc.dram_tensor("bi", [PT, NOUT], MM_DT)
        out = nc.dram_tensor("out", [BS, NOUT], mybir.dt.float32)
        tok = nc.dram_tensor("tok", [1, 1], mybir.dt.float32,
                             kind="ExternalOutput")
    else:
        st = nc.dram_tensor("st", [MT, PT, KT * PT], MM_DT,
                            kind="ExternalInput")
        w = nc.dram_tensor("w", [KT, PT, NOUT], MM_DT,
                           kind="ExternalInput")
        bi = nc.dram_tensor("bi", [PT, NOUT], MM_DT,
                            kind="ExternalInput")
        out = nc.dram_tensor("out", [BS, NOUT], mybir.dt.float32,
                             kind="ExternalOutput")
        tok = None

    with tile.TileContext(nc) as tc:
        with (
            tc.tile_pool(name="wpool", bufs=1) as wpool,
            tc.tile_pool(name="spool", bufs=4) as spool,
            tc.tile_pool(name="opool", bufs=4) as opool,
            tc.tile_pool(name="ppool", bufs=8, space="PSUM") as ppool,
        ):
            queues = {"act": nc.scalar, "pool": nc.gpsimd, "sp": nc.sync}
            if chunked_w:
                w0p = [wpool.tile([PT, (c1 - c0) * NFREE], MM_DT,
                                  name=f"wt0p{i}", tag=f"wt0p{i}")
                       for i, (c0, c1, _) in enumerate(P0)]
            wbig = [None if (chunked_w and k == 0) else
                    wpool.tile([PT, NOUT], MM_DT, name=f"wt{k}",
                               tag=f"wt{k}") for k in range(KT)]

            def w0chunk(n):
                for i, (c0, c1, _) in enumerate(P0):
                    if c0 <= n < c1:
                        return w0p[i][:, (n - c0) * NFREE:
                                      (n - c0 + 1) * NFREE]

            def wchunk(k, n):
                if chunked_w and k == 0:
                    return w0chunk(n)
                return wbig[k][:, n * NFREE:(n + 1) * NFREE]

            bt = wpool.tile([PT, NOUT], MM_DT, name="bt", tag="bt")
            bias_loaded = [False]

            def issue_bias():
                nc.sync.dma_start(bt[:], bi[:])
                bias_loaded[0] = True

            def issue_sp_w():
                for i, (c0, c1, q) in enumerate(P0):
                    if q == "sp":
                        queues[q].dma_start(
                            w0p[i][:], w[0][:, c0 * NFREE:c1 * NFREE])
                for k, q in W_WHOLE_Q:
                    if q == "sp":
                        queues[q].dma_start(wbig[k][:], w[k])

            if chunked_w:
                for i, (c0, c1, q) in enumerate(P0):
                    if q != "sp":
                        queues[q].dma_start(
                            w0p[i][:], w[0][:, c0 * NFREE:c1 * NFREE])
                for k, q in W_WHOLE_Q:
                    if q != "sp":
                        queues[q].dma_start(wbig[k][:], w[k])
                sp_pending = [1]
            else:
                for k in range(KT):
                    queues[("act", "pool")[k % 2]].dma_start(
                        wbig[k][:], w[k])
                sp_pending = []

            if timing and repeat > 1:
                if sp_pending:
                    issue_sp_w()
                    sp_pending = []
                issue_bias()

            def body():
                pre_s1 = [None]
                for m in range(MT):
                    src = st[:] if timing else st[m]
                    if m == 0 and sp_pending:
                        s_a = spool.tile([PT, PT], MM_DT, name="s0a",
                                         tag="s0a")
                        s_b = spool.tile([PT, (KT - 1) * PT], MM_DT,
                                         name="s0b", tag="s0b")
                        nc.sync.dma_start(s_a[:], src[:, 0:PT])
                        nc.sync.dma_start(s_b[:], src[:, PT:])
                        issue_sp_w()
                        sp_pending.clear()
                        s1_t = spool.tile([PT, KT * PT], MM_DT,
                                          name="s1", tag="s")
                        nc.sync.dma_start(s1_t[:],
                                          st[:] if timing else st[1])
                        pre_s1[0] = s1_t
                        if not bias_loaded[0]:
                            issue_bias()
                        sts = [s_a[:] if k == 0 else
                               s_b[:, (k - 1) * PT:k * PT]
                               for k in range(KT)]
                    elif m == 1 and pre_s1[0] is not None:
                        s_t = pre_s1[0]
                        sts = [s_t[:, k * PT:(k + 1) * PT]
                               for k in range(KT)]
                    else:
                        s_t = spool.tile([PT, KT * PT], MM_DT,
                                         name=f"s{m}", tag="s")
                        nc.sync.dma_start(s_t[:], src)
                        if not bias_loaded[0]:
                            issue_bias()
                        sts = [s_t[:, k * PT:(k + 1) * PT]
                               for k in range(KT)]
                    ot = opool.tile([PT, NOUT], mybir.dt.float32,
                                    name=f"o{m}", tag="o")
                    last_m = m == MT - 1
                    n_pts = NT - 1 if (last_m and split_tail) else NT
                    pts = [ppool.tile([PT, NFREE], mybir.dt.float32,
                                      name=f"p{m}_{n}", tag="p")
                           for n in range(n_pts)]
                    rows = slice(m * PT, (m + 1) * PT)
                    store_q = queues["act" if m % 2 == 0 else "pool"]

                    def mm(k, n):
                        nc.tensor.matmul(
                            pts[n][:], sts[k], wchunk(k, n),
                            start=(k == 0), stop=(k == KT - 1),
                        )

                    def drain(n):
                        nsl = slice(n * NFREE, (n + 1) * NFREE)
                        nc.vector.tensor_add(ot[:, nsl], pts[n][:],
                                             bt[:, nsl])

                    if m == 0:
                        for k in range(KT):
                            for n in range(NT):
                                mm(k, n)
                        for n in range(NT):
                            drain(n)
                        store_q.dma_start(out[rows, :], ot[:])
                        continue

                    for n in range(NT):
                        nsl = slice(n * NFREE, (n + 1) * NFREE)
                        if last_m and n == NT - 1 and split_tail:
                            ha = 3 * NFREE // 4
                            sa = slice(n * NFREE, n * NFREE + ha)
                            sb = slice(n * NFREE + ha, (n + 1) * NFREE)
                            p5a = ppool.tile([PT, ha], mybir.dt.float32,
                                             name="p5a", tag="p")
                            p5b = ppool.tile([PT, NFREE - ha],
                                             mybir.dt.float32,
                                             name="p5b", tag="p")
                            wc = [wchunk(k, n) for k in range(KT)]
                            for k in range(KT):
                                nc.tensor.matmul(
                                    p5a[:], sts[k], wc[k][:, 0:ha],
                                    start=(k == 0), stop=(k == KT - 1))
                            o_a = opool.tile([PT, ha], mybir.dt.float32,
                                             name="o_a", tag="o_a")
                            nc.vector.tensor_add(o_a[:], p5a[:], bt[:, sa])
                            nc.gpsimd.dma_start(out[rows, sa], o_a[:])
                            for k in range(KT):
                                nc.tensor.matmul(
                                    p5b[:], sts[k], wc[k][:, ha:],
                                    start=(k == 0), stop=(k == KT - 1))
                            o_b = opool.tile([PT, NFREE - ha],
                                             mybir.dt.float32,
                                             name="o_b", tag="o_b")
                            nc.vector.tensor_add(o_b[:], p5b[:], bt[:, sb])
                            nc.scalar.dma_start(out[rows, sb], o_b[:])
                            continue
                        for k in range(KT):
                            mm(k, n)
                        if not last_m:
                            drain(n)
                            if n == NT - 1:
                                store_q.dma_start(out[rows, :], ot[:])
                        else:
                            drain(n)
                            nc.gpsimd.dma_start(out[rows, nsl], ot[:, nsl])

            if repeat > 1:
                with tc.For_i(0, repeat, 1,
                              hint_engines=(mybir.EngineType.PE,)):
                    body()
            else:
                body()
            if tok is not None:
                tk = wpool.tile([1, 1], mybir.dt.float32, name="tk", tag="tk")
                nc.gpsimd.memset(tk[:], 1.0)
                nc.sync.dma_start(tok[:], tk[:])

    nc.compile()
    return nc


def _fold_params(fuse_w, fuse_b, t_in_w, t_in_b, t_out_w, t_out_b,
                 i_in_w, i_in_b, i_out_w, i_out_b):
    f8 = np.float64
    fuse_w8, fuse_b8 = fuse_w.astype(f8), fuse_b.astype(f8)

    def fold(in_w, in_b, out_w, out_b):
        wv = in_w[2 * D:3 * D].astype(f8)
        bv = in_b[2 * D:3 * D].astype(f8)
        Wm = out_w.astype(f8) @ wv
        bm = out_w.astype(f8) @ bv + out_b.astype(f8)
        W2 = Wm @ fuse_w8
        b2 = Wm @ fuse_b8 + bm
        return W2, b2

    Wt2, bias_t = fold(t_in_w, t_in_b, t_out_w, t_out_b)
    Wi2, bias_i = fold(i_in_w, i_in_b, i_out_w, i_out_b)

    W_all = np.empty((D, NOUT), np.float32)
    W_all[:, 0:D] = fuse_w8.T
    W_all[:, D:2 * D] = Wt2.T
    W_all[:, 2 * D:3 * D] = Wi2.T
    bias_all = np.empty(NOUT, np.float32)
    bias_all[0:D] = fuse_b
    bias_all[D:2 * D] = bias_t
    bias_all[2 * D:3 * D] = bias_i
    return W_all, bias_all


def kernel(text_feat, image_feat, fuse_w, fuse_b,
           t_in_w, t_in_b, t_out_w, t_out_b,
           i_in_w, i_in_b, i_out_w, i_out_b):
    global _COMPILED, LAST_RESULTS
    text_feat = np.asarray(text_feat, np.float32)
    image_feat = np.asarray(image_feat, np.float32)
    args = [np.asarray(a, np.float32) for a in
            (fuse_w, fuse_b, t_in_w, t_in_b, t_out_w, t_out_b,
             i_in_w, i_in_b, i_out_w, i_out_b)]
    W_all, bias_all = _fold_params(*args)

    S = text_feat + image_feat
    in_maps = []
    w_arr = np.ascontiguousarray(W_all.reshape(KT, PT, NOUT).astype(NP_DT))
    bi_arr = np.ascontiguousarray(
        np.broadcast_to(bias_all.astype(NP_DT), (PT, NOUT)))
    for c in range(NCORES):
        Sc = S[c * BS:(c + 1) * BS]
        stc = np.ascontiguousarray(
            Sc.reshape(MT, PT, KT, PT).transpose(0, 3, 2, 1)
            .reshape(MT, PT, KT * PT).astype(NP_DT))
        in_maps.append({"st": stc, "w": w_arr, "bi": bi_arr})

    if _COMPILED is None:
        _COMPILED = _build()

    LAST_RESULTS = run_bass_kernel_spmd(
        _COMPILED, in_maps, core_ids=list(range(NCORES)))
    outs = np.concatenate([r["out"] for r in LAST_RESULTS.results], axis=0)

    fused = outs[:, 0:D]
    out_text = outs[:, D:2 * D]
    out_image = outs[:, 2 * D:3 * D]
    return (out_text, out_image, fused)
